# revision 33
# baseline (speedup 1.0000x reference)
"""CrossAttentionFusion Trainium2 kernel (nn_CrossAttentionFusion__45561013076033).

Full inputs -> full output. Sharding: 8 cores, core c handles batch b=c//2,
query-half h=c%2 (2048 of 4096 queries). Each core holds the full [256,4096]
cnn feature map of its batch (keys/values), its query-half of the transformer
features, and replicated weights.

v7 dataflow:
  - fp16 projections (Q/K/V written as fp8e4m3), fp16 fused conv
  - scores computed transposed (S^T tiles [key,query]) via fp8 DoubleRow
    matmuls (full 256-deep contraction per matmul); exp writes P^T fp8
    straight from PSUM
  - rowsums via fp8 DoubleRow ones-matmuls (replicated across partitions),
    fast-approx reciprocal, normalization folded into the attended output
  - scores / rowsum / attended matmuls interleaved per key-tile-pair within
    each 512-query superblock (consumers lag producers by one pair) so the
    PE never waits on the ACT exp stream
  - PE warm-up matmul chain at t~0 to open the HAM clock gate early
"""

import numpy as np

B, C, H, W = 4, 256, 64, 64
N = H * W            # 4096 tokens
NCORES = 8
QH = N // 2          # 2048 queries per core
CT = C // 128        # 2 channel tiles
KC = N // 512        # 8 key chunks of 512
NSB = QH // 512      # 4 query superblocks per core
NKT = N // 128       # 32 key tiles
NG = NKT // 2        # 16 key-tile pairs

_CACHE = {}


def _build():
    import concourse.bass as bass
    import concourse.mybir as mybir
    import concourse.tile as tile
    from concourse import bacc

    f32 = mybir.dt.float32
    f16 = mybir.dt.float16
    f8 = mybir.dt.float8e4
    DR = mybir.MatmulPerfMode.DoubleRow
    AF = mybir.ActivationFunctionType

    nc = bacc.Bacc("TRN2", target_bir_lowering=False, debug=True)

    WARM = nc.dram_tensor("warm", [128, 64], f32, kind="ExternalOutput")
    XQ = nc.dram_tensor("xq", [C, QH], f16, kind="ExternalInput")
    XC = nc.dram_tensor("xc", [C, N], f16, kind="ExternalInput")
    WQT = nc.dram_tensor("wqt", [C, C], f16, kind="ExternalInput")
    WKT = nc.dram_tensor("wkt", [C, C], f16, kind="ExternalInput")
    WVT = nc.dram_tensor("wvt", [C, C], f16, kind="ExternalInput")
    WFT = nc.dram_tensor("wft", [2 * C, C], f16, kind="ExternalInput")
    BQ = nc.dram_tensor("bq", [C], f32, kind="ExternalInput")
    BK = nc.dram_tensor("bk", [C], f32, kind="ExternalInput")
    BF = nc.dram_tensor("bf", [C], f32, kind="ExternalInput")
    OUT = nc.dram_tensor("out", [C, QH], f32, kind="ExternalOutput")

    xq_d = XQ.ap().rearrange("(t p) n -> p t n", p=128)
    xc_d = XC.ap().rearrange("(t p) n -> p t n", p=128)
    wq_d = WQT.ap().rearrange("(t p) d -> p t d", p=128)
    wk_d = WKT.ap().rearrange("(t p) d -> p t d", p=128)
    wv_d = WVT.ap().rearrange("(t p) d -> p t d", p=128)
    wf_d = WFT.ap().rearrange("(t p) d -> p t d", p=128)
    out_d = OUT.ap().rearrange("(t p) n -> p t n", p=128)

    with tile.TileContext(nc) as tc:
        with tc.tile_pool(name="persist", bufs=1) as per, \
             tc.tile_pool(name="ptp", bufs=2) as ptp, \
             tc.tile_pool(name="tree", bufs=1) as trp, \
             tc.tile_pool(name="norm", bufs=2) as nrp, \
             tc.tile_pool(name="outp", bufs=2) as outp, \
             tc.tile_pool(name="sc", bufs=2, space="PSUM") as scp, \
             tc.tile_pool(name="av", bufs=2, space="PSUM") as avp:

            # ---- persistent tiles ----
            xq_sb = per.tile([128, CT, QH], f16)
            xc_sb = per.tile([128, CT, N], f16)
            wq_sb = per.tile([128, CT, C], f16)
            wk_sb = per.tile([128, CT, C], f16)
            wv_sb = per.tile([128, CT, C], f16)
            wf_sb = per.tile([128, 2 * CT, C], f16)
            bq_sb = per.tile([128, CT], f32)
            bk_sb = per.tile([128, CT], f32)
            bf_sb = per.tile([128, CT], f32)
            q_sb = per.tile([128, CT, QH], f8)
            k_sb = per.tile([128, CT, N], f8)
            vt_sb = per.tile([128, NKT, C], f8)
            ones_sb = per.tile([128, 2, 128], f8)
            onesw_sb = per.tile([128, 128], f16)

            nc.sync.dma_start(wq_sb[:], wq_d)
            nc.sync.dma_start(wk_sb[:], wk_d)
            nc.sync.dma_start(wv_sb[:], wv_d)
            nc.sync.dma_start(wf_sb[:], wf_d)
            nc.sync.dma_start(bq_sb[:], BQ.ap().rearrange("(t p) -> p t", p=128))
            nc.sync.dma_start(bk_sb[:], BK.ap().rearrange("(t p) -> p t", p=128))
            nc.sync.dma_start(bf_sb[:], BF.ap().rearrange("(t p) -> p t", p=128))
            nc.vector.memset(ones_sb[:], 1.0)
            nc.vector.memset(onesw_sb[:], 1.0)

            # ---- PE warm-up: keep the PE busy from t~0 so the HAM clock
            # gate opens (1.2 -> 2.4 GHz) before the real matmuls start.
            wps = scp.tile([128, 512], f32, tag="sc")
            for _ in range(120):
                nc.tensor.matmul(wps[:, :64], onesw_sb[:], onesw_sb[:, :64],
                                 start=True, stop=True)
            w_sb = per.tile([128, 64], f32)
            nc.scalar.activation(w_sb[:], wps[:, :64], AF.Copy)
            nc.sync.dma_start(WARM.ap(), w_sb[:])

            # ---- inputs + projections, interleaved per 512-column chunk ----
            # Q projection: Q[d, n], bias added on DVE
            for qc in range(QH // 512):
                s = slice(qc * 512, (qc + 1) * 512)
                for ct in range(CT):
                    nc.sync.dma_start(xq_sb[:, ct, s], xq_d[:, ct, s])
                for dt in range(CT):
                    ps = scp.tile([128, 512], f32, tag="sc")
                    for ct in range(CT):
                        nc.tensor.matmul(
                            ps[:], wq_sb[:, ct, dt * 128:(dt + 1) * 128],
                            xq_sb[:, ct, s],
                            start=(ct == 0), stop=(ct == CT - 1))
                    nc.vector.tensor_scalar_add(
                        q_sb[:, dt, s], ps[:], bq_sb[:, dt:dt + 1])

            # K + V^T projections per xc chunk
            for kc in range(KC):
                s = slice(kc * 512, (kc + 1) * 512)
                for ct in range(CT):
                    nc.sync.dma_start(xc_sb[:, ct, s], xc_d[:, ct, s])
                for dt in range(CT):
                    ps = scp.tile([128, 512], f32, tag="sc")
                    for ct in range(CT):
                        nc.tensor.matmul(
                            ps[:], wk_sb[:, ct, dt * 128:(dt + 1) * 128],
                            xc_sb[:, ct, s],
                            start=(ct == 0), stop=(ct == CT - 1))
                    nc.vector.tensor_scalar_add(
                        k_sb[:, dt, s], ps[:], bk_sb[:, dt:dt + 1])
                # V^T: [keys, d] (no bias; folded into conv bias)
                for j in range(4):
                    kt = 4 * kc + j
                    ps = scp.tile([128, 512], f32, tag="sc")
                    for ct in range(CT):
                        nc.tensor.matmul(
                            ps[:, :C], xc_sb[:, ct, kt * 128:(kt + 1) * 128],
                            wv_sb[:, ct],
                            start=(ct == 0), stop=(ct == CT - 1))
                    nc.scalar.activation(vt_sb[:, kt], ps[:, :C], AF.Copy)

            # ---- attention + fused conv, per 512-query superblock ----
            # Per key-tile pair g: two fp8-DR score matmuls + exps feed the
            # rowsum + attended matmuls of pair g-1 (one-pair lag keeps the
            # PE from waiting on ACT). Previous superblock's normalize/conv
            # are emitted after this superblock's first pairs.
            nc._v7_tiles = (outp, avp, wf_sb, xq_sb, bf_sb, out_d)
            prev = None
            for sb in range(NSB):
                qs = slice(sb * 512, (sb + 1) * 512)
                pt = ptp.tile([128, NKT, 512], f8, tag="pt")
                aps = avp.tile([128, 2, 512], f32, tag="av")
                acc = trp.tile([128, 512], f16, tag="acc")

                def scores_pair(g):
                    ps = scp.tile([128, 1024], f32, tag="sc")
                    for j in range(2):
                        kt = 2 * g + j
                        nc.tensor.matmul(
                            ps[:, j * 512:(j + 1) * 512],
                            k_sb[:, :, kt * 128:(kt + 1) * 128],
                            q_sb[:, :, qs],
                            start=True, stop=True, perf_mode=DR)
                    nc.scalar.activation(pt[:, 2 * g:2 * (g + 1)], ps[:],
                                         AF.Exp)
                    # incremental partition-partial rowsum on DVE
                    if g == 0:
                        nc.vector.tensor_add(acc[:], pt[:, 0], pt[:, 1])
                    else:
                        tmp = trp.tile([128, 512], f16, tag="tmp")
                        nc.vector.tensor_add(tmp[:], pt[:, 2 * g],
                                             pt[:, 2 * g + 1])
                        nc.vector.tensor_add(acc[:], acc[:], tmp[:])

                def consume_pair(g):
                    for dt in range(CT):
                        nc.tensor.matmul(
                            aps[:, dt],
                            vt_sb[:, 2 * g:2 * (g + 1),
                                  dt * 128:(dt + 1) * 128],
                            pt[:, 2 * g:2 * (g + 1)],
                            start=(g == 0), stop=(g == NG - 1),
                            perf_mode=DR)

                scores_pair(0)
                scores_pair(1)
                if prev is not None:
                    _finish_sb(nc, mybir, prev)
                for g in range(2, NG):
                    scores_pair(g)
                    consume_pair(g - 2)
                consume_pair(NG - 2)
                consume_pair(NG - 1)

                # partition-reduce + replicate via one fp16 ones-matmul
                rs = scp.tile([128, 1024], f32, tag="sc")
                nc.tensor.matmul(rs[:, :512], onesw_sb[:], acc[:],
                                 start=True, stop=True)
                rinv = nrp.tile([128, 512], f32, tag="rinv")
                nc.vector.reciprocal_approx_fast(rinv[:], rs[:, :512])
                prev = dict(qs=qs, aps=aps, rinv=rinv)

            _finish_sb(nc, mybir, prev)
    nc.finalize()
    return nc


def _finish_sb(nc, mybir, env):
    """Normalize attended output, fused 1x1 conv, bias, DMA out."""
    f32 = mybir.dt.float32
    f16 = mybir.dt.float16
    tiles = nc._v7_tiles
    qs, aps, rinv = env["qs"], env["aps"], env["rinv"]
    (outp, avp, wf_sb, xq_sb, bf_sb, out_d) = tiles

    a_sb = outp.tile([128, CT, 512], f16, tag="a")
    for dt in range(CT):
        nc.vector.tensor_mul(a_sb[:, dt], aps[:, dt], rinv[:])
    o_sb = outp.tile([128, CT, 512], f32, tag="o")
    cv = avp.tile([128, 2, 512], f32, tag="av")
    for dt in range(CT):
        for j in range(2 * CT):
            rhs = xq_sb[:, j, qs] if j < CT else a_sb[:, j - CT]
            nc.tensor.matmul(
                cv[:, dt], wf_sb[:, j, dt * 128:(dt + 1) * 128],
                rhs, start=(j == 0), stop=(j == 2 * CT - 1))
        nc.vector.tensor_scalar_add(
            o_sb[:, dt], cv[:, dt], bf_sb[:, dt:dt + 1])
        nc.sync.dma_start(out_d[:, dt, qs], o_sb[:, dt])


def _get_nc():
    if "nc" not in _CACHE:
        _CACHE["nc"] = _build()
    return _CACHE["nc"]


def _in_maps(transformer_features, cnn_features, Wq, bq, Wk, bk, Wv, bv, Wf, bf):
    xt = np.ascontiguousarray(np.asarray(transformer_features, np.float32)
                              .reshape(B, C, N))
    xc = np.ascontiguousarray(np.asarray(cnn_features, np.float32)
                              .reshape(B, C, N))
    Wq = np.asarray(Wq, np.float32)
    Wk = np.asarray(Wk, np.float32)
    Wv = np.asarray(Wv, np.float32)
    Wf = np.asarray(Wf, np.float32)
    bq = np.asarray(bq, np.float32)
    bk = np.asarray(bk, np.float32)
    bv = np.asarray(bv, np.float32)
    bf = np.asarray(bf, np.float32)

    wqt = np.ascontiguousarray(Wq.T / 16.0).astype(np.float16)
    wkt = np.ascontiguousarray(Wk.T).astype(np.float16)
    wvt = np.ascontiguousarray(Wv.T).astype(np.float16)
    wft = np.ascontiguousarray(Wf.T).astype(np.float16)
    bq_s = bq / 16.0
    bf2 = bf + Wf[:, C:] @ bv
    xt16 = xt.astype(np.float16)
    xc16 = xc.astype(np.float16)

    maps = []
    for c in range(NCORES):
        b, h = divmod(c, 2)
        maps.append(dict(
            xq=np.ascontiguousarray(xt16[b][:, h * QH:(h + 1) * QH]),
            xc=xc16[b],
            wqt=wqt, wkt=wkt, wvt=wvt, wft=wft,
            bq=bq_s, bk=bk, bf=bf2,
        ))
    return maps


def _run(inputs, trace=False):
    from concourse.bass_utils import run_bass_kernel_spmd
    nc = _get_nc()
    maps = _in_maps(**inputs)
    return run_bass_kernel_spmd(nc, maps, list(range(NCORES)), trace=trace)


def kernel(**inputs) -> np.ndarray:
    res = _run(inputs).results
    out = np.empty((B, C, N), np.float32)
    for c in range(NCORES):
        b, h = divmod(c, 2)
        out[b][:, h * QH:(h + 1) * QH] = res[c]["out"]
    return out.reshape(B, C, H, W)


# revision 37
# speedup vs baseline: 1.0870x; 1.0870x over previous
"""CrossAttentionFusion Trainium2 kernel (nn_CrossAttentionFusion__45561013076033).

Full inputs -> full output. Sharding: 8 cores, core c handles batch b=c//2,
query-half h=c%2 (2048 of 4096 queries). Each core holds the full [256,4096]
cnn feature map of its batch (keys/values), its query-half of the transformer
features, and replicated weights.

v7 dataflow:
  - fp16 projections (Q/K/V written as fp8e4m3), fp16 fused conv
  - scores computed transposed (S^T tiles [key,query]) via fp8 DoubleRow
    matmuls (full 256-deep contraction per matmul); exp writes P^T fp8
    straight from PSUM
  - rowsums via fp8 DoubleRow ones-matmuls (replicated across partitions),
    fast-approx reciprocal, normalization folded into the attended output
  - scores / rowsum / attended matmuls interleaved per key-tile-pair within
    each 512-query superblock (consumers lag producers by one pair) so the
    PE never waits on the ACT exp stream
  - PE warm-up matmul chain at t~0 to open the HAM clock gate early
"""

import numpy as np

B, C, H, W = 4, 256, 64, 64
N = H * W            # 4096 tokens
NCORES = 8
QH = N // 2          # 2048 queries per core
CT = C // 128        # 2 channel tiles
KC = N // 512        # 8 key chunks of 512
NSB = QH // 512      # 4 query superblocks per core
NKT = N // 128       # 32 key tiles
NG = NKT // 2        # 16 key-tile pairs

_CACHE = {}


def _build():
    import concourse.bass as bass
    import concourse.mybir as mybir
    import concourse.tile as tile
    from concourse import bacc

    f32 = mybir.dt.float32
    f16 = mybir.dt.float16
    f8 = mybir.dt.float8e4
    DR = mybir.MatmulPerfMode.DoubleRow
    AF = mybir.ActivationFunctionType

    nc = bacc.Bacc("TRN2", target_bir_lowering=False, debug=True)

    WARM = nc.dram_tensor("warm", [128, 64], f32, kind="ExternalOutput")
    XQ = nc.dram_tensor("xq", [C, QH], f16, kind="ExternalInput")
    XC = nc.dram_tensor("xc", [C, N], f16, kind="ExternalInput")
    WQT = nc.dram_tensor("wqt", [C, C], f16, kind="ExternalInput")
    WKT = nc.dram_tensor("wkt", [C, C], f16, kind="ExternalInput")
    WVT = nc.dram_tensor("wvt", [C, C], f16, kind="ExternalInput")
    WFT = nc.dram_tensor("wft", [2 * C, C], f16, kind="ExternalInput")
    BQ = nc.dram_tensor("bq", [C], f32, kind="ExternalInput")
    BK = nc.dram_tensor("bk", [C], f32, kind="ExternalInput")
    BF = nc.dram_tensor("bf", [C], f32, kind="ExternalInput")
    OUT = nc.dram_tensor("out", [C, QH], f32, kind="ExternalOutput")

    xq_d = XQ.ap().rearrange("(t p) n -> p t n", p=128)
    xc_d = XC.ap().rearrange("(t p) n -> p t n", p=128)
    wq_d = WQT.ap().rearrange("(t p) d -> p t d", p=128)
    wk_d = WKT.ap().rearrange("(t p) d -> p t d", p=128)
    wv_d = WVT.ap().rearrange("(t p) d -> p t d", p=128)
    wf_d = WFT.ap().rearrange("(t p) d -> p t d", p=128)
    out_d = OUT.ap().rearrange("(t p) n -> p t n", p=128)

    with tile.TileContext(nc) as tc:
        with tc.tile_pool(name="persist", bufs=1) as per, \
             tc.tile_pool(name="ptp", bufs=2) as ptp, \
             tc.tile_pool(name="norm", bufs=2) as nrp, \
             tc.tile_pool(name="outp", bufs=2) as outp, \
             tc.tile_pool(name="sc", bufs=2, space="PSUM") as scp, \
             tc.tile_pool(name="rs", bufs=2, space="PSUM") as rsp, \
             tc.tile_pool(name="av", bufs=2, space="PSUM") as avp:

            # ---- persistent tiles ----
            xq_sb = per.tile([128, CT, QH], f16)
            xc_sb = per.tile([128, CT, N], f16)
            wq_sb = per.tile([128, CT, C], f16)
            wk_sb = per.tile([128, CT, C], f16)
            wv_sb = per.tile([128, CT, C], f16)
            wf_sb = per.tile([128, 2 * CT, C], f16)
            bq_sb = per.tile([128, CT], f32)
            bk_sb = per.tile([128, CT], f32)
            bf_sb = per.tile([128, CT], f32)
            q_sb = per.tile([128, CT, QH], f8)
            k_sb = per.tile([128, CT, N], f8)
            vt_sb = per.tile([128, NKT, C], f8)
            ones_sb = per.tile([128, 2, 128], f8)
            onesw_sb = per.tile([128, 128], f16)

            nc.sync.dma_start(wq_sb[:], wq_d)
            nc.sync.dma_start(wk_sb[:], wk_d)
            nc.sync.dma_start(wv_sb[:], wv_d)
            nc.sync.dma_start(wf_sb[:], wf_d)
            nc.sync.dma_start(bq_sb[:], BQ.ap().rearrange("(t p) -> p t", p=128))
            nc.sync.dma_start(bk_sb[:], BK.ap().rearrange("(t p) -> p t", p=128))
            nc.sync.dma_start(bf_sb[:], BF.ap().rearrange("(t p) -> p t", p=128))
            nc.vector.memset(ones_sb[:], 1.0)
            nc.vector.memset(onesw_sb[:], 1.0)

            # ---- PE warm-up: keep the PE busy from t~0 so the HAM clock
            # gate opens (1.2 -> 2.4 GHz) before the real matmuls start.
            wps = scp.tile([128, 512], f32, tag="sc")
            for _ in range(120):
                nc.tensor.matmul(wps[:, :64], onesw_sb[:], onesw_sb[:, :64],
                                 start=True, stop=True)
            w_sb = per.tile([128, 64], f32)
            nc.scalar.activation(w_sb[:], wps[:, :64], AF.Copy)
            nc.sync.dma_start(WARM.ap(), w_sb[:])

            # ---- inputs + projections, interleaved per 512-column chunk ----
            # Q projection: Q[d, n], bias added on DVE
            for qc in range(QH // 512):
                s = slice(qc * 512, (qc + 1) * 512)
                for ct in range(CT):
                    nc.sync.dma_start(xq_sb[:, ct, s], xq_d[:, ct, s])
                for dt in range(CT):
                    ps = scp.tile([128, 512], f32, tag="sc")
                    for ct in range(CT):
                        nc.tensor.matmul(
                            ps[:], wq_sb[:, ct, dt * 128:(dt + 1) * 128],
                            xq_sb[:, ct, s],
                            start=(ct == 0), stop=(ct == CT - 1))
                    nc.vector.tensor_scalar_add(
                        q_sb[:, dt, s], ps[:], bq_sb[:, dt:dt + 1])

            # K + V^T projections per xc chunk
            for kc in range(KC):
                s = slice(kc * 512, (kc + 1) * 512)
                for ct in range(CT):
                    nc.sync.dma_start(xc_sb[:, ct, s], xc_d[:, ct, s])
                for dt in range(CT):
                    ps = scp.tile([128, 512], f32, tag="sc")
                    for ct in range(CT):
                        nc.tensor.matmul(
                            ps[:], wk_sb[:, ct, dt * 128:(dt + 1) * 128],
                            xc_sb[:, ct, s],
                            start=(ct == 0), stop=(ct == CT - 1))
                    nc.vector.tensor_scalar_add(
                        k_sb[:, dt, s], ps[:], bk_sb[:, dt:dt + 1])
                # V^T: [keys, d] (no bias; folded into conv bias)
                for j in range(4):
                    kt = 4 * kc + j
                    ps = scp.tile([128, 512], f32, tag="sc")
                    for ct in range(CT):
                        nc.tensor.matmul(
                            ps[:, :C], xc_sb[:, ct, kt * 128:(kt + 1) * 128],
                            wv_sb[:, ct],
                            start=(ct == 0), stop=(ct == CT - 1))
                    nc.scalar.activation(vt_sb[:, kt], ps[:, :C], AF.Copy)

            # ---- attention + fused conv, per 512-query superblock ----
            # Per key-tile pair g: two fp8-DR score matmuls + exps feed the
            # rowsum + attended matmuls of pair g-1 (one-pair lag keeps the
            # PE from waiting on ACT). Previous superblock's normalize/conv
            # are emitted after this superblock's first pairs.
            nc._v7_tiles = (outp, rsp, wf_sb, xq_sb, bf_sb, out_d)
            prev = None
            for sb in range(NSB):
                qs = slice(sb * 512, (sb + 1) * 512)
                pt = ptp.tile([128, NKT, 512], f8, tag="pt")
                rs = rsp.tile([128, 512], f32, tag="rs")
                aps = avp.tile([128, 2, 512], f32, tag="av")

                def scores_pair(g):
                    for j in range(2):
                        kt = 2 * g + j
                        ps = scp.tile([128, 512], f32, tag="sc")
                        nc.tensor.matmul(
                            ps[:], k_sb[:, :, kt * 128:(kt + 1) * 128],
                            q_sb[:, :, qs],
                            start=True, stop=True, perf_mode=DR)
                        nc.scalar.activation(pt[:, kt], ps[:], AF.Exp)

                def consume_pair(g):
                    nc.tensor.matmul(
                        rs[:], ones_sb[:], pt[:, 2 * g:2 * (g + 1)],
                        start=(g == 0), stop=(g == NG - 1), perf_mode=DR)
                    for dt in range(CT):
                        nc.tensor.matmul(
                            aps[:, dt],
                            vt_sb[:, 2 * g:2 * (g + 1),
                                  dt * 128:(dt + 1) * 128],
                            pt[:, 2 * g:2 * (g + 1)],
                            start=(g == 0), stop=(g == NG - 1),
                            perf_mode=DR)

                scores_pair(0)
                scores_pair(1)
                if prev is not None:
                    _finish_sb(nc, mybir, prev)
                    prev = None
                for g in range(2, NG):
                    scores_pair(g)
                    consume_pair(g - 2)
                consume_pair(NG - 2)
                consume_pair(NG - 1)

                rinv = nrp.tile([128, 512], f32, tag="rinv")
                nc.vector.reciprocal_approx_fast(rinv[:], rs[:])
                prev = dict(qs=qs, aps=aps, rinv=rinv)

            _finish_sb(nc, mybir, prev)
    nc.finalize()
    return nc


def _finish_sb(nc, mybir, env):
    """Normalize attended output, fused 1x1 conv, bias, DMA out."""
    f32 = mybir.dt.float32
    f16 = mybir.dt.float16
    tiles = nc._v7_tiles
    qs, aps, rinv = env["qs"], env["aps"], env["rinv"]
    (outp, rsp, wf_sb, xq_sb, bf_sb, out_d) = tiles

    a_sb = outp.tile([128, CT, 512], f16, tag="a")
    for dt in range(CT):
        nc.vector.tensor_mul(a_sb[:, dt], aps[:, dt], rinv[:])
    o_sb = outp.tile([128, CT, 512], f32, tag="o")
    for dt in range(CT):
        ops = rsp.tile([128, 512], f32, tag="rs")
        for j in range(2 * CT):
            rhs = xq_sb[:, j, qs] if j < CT else a_sb[:, j - CT]
            nc.tensor.matmul(
                ops[:], wf_sb[:, j, dt * 128:(dt + 1) * 128],
                rhs, start=(j == 0), stop=(j == 2 * CT - 1))
        nc.vector.tensor_scalar_add(
            o_sb[:, dt], ops[:], bf_sb[:, dt:dt + 1])
        nc.sync.dma_start(out_d[:, dt, qs], o_sb[:, dt])


def _get_nc():
    if "nc" not in _CACHE:
        _CACHE["nc"] = _build()
    return _CACHE["nc"]


def _in_maps(transformer_features, cnn_features, Wq, bq, Wk, bk, Wv, bv, Wf, bf):
    xt = np.ascontiguousarray(np.asarray(transformer_features, np.float32)
                              .reshape(B, C, N))
    xc = np.ascontiguousarray(np.asarray(cnn_features, np.float32)
                              .reshape(B, C, N))
    Wq = np.asarray(Wq, np.float32)
    Wk = np.asarray(Wk, np.float32)
    Wv = np.asarray(Wv, np.float32)
    Wf = np.asarray(Wf, np.float32)
    bq = np.asarray(bq, np.float32)
    bk = np.asarray(bk, np.float32)
    bv = np.asarray(bv, np.float32)
    bf = np.asarray(bf, np.float32)

    wqt = np.ascontiguousarray(Wq.T / 16.0).astype(np.float16)
    wkt = np.ascontiguousarray(Wk.T).astype(np.float16)
    wvt = np.ascontiguousarray(Wv.T).astype(np.float16)
    wft = np.ascontiguousarray(Wf.T).astype(np.float16)
    bq_s = bq / 16.0
    bf2 = bf + Wf[:, C:] @ bv
    xt16 = xt.astype(np.float16)
    xc16 = xc.astype(np.float16)

    maps = []
    for c in range(NCORES):
        b, h = divmod(c, 2)
        maps.append(dict(
            xq=np.ascontiguousarray(xt16[b][:, h * QH:(h + 1) * QH]),
            xc=xc16[b],
            wqt=wqt, wkt=wkt, wvt=wvt, wft=wft,
            bq=bq_s, bk=bk, bf=bf2,
        ))
    return maps


def _run(inputs, trace=False):
    from concourse.bass_utils import run_bass_kernel_spmd
    nc = _get_nc()
    maps = _in_maps(**inputs)
    return run_bass_kernel_spmd(nc, maps, list(range(NCORES)), trace=trace)


def kernel(**inputs) -> np.ndarray:
    res = _run(inputs).results
    out = np.empty((B, C, N), np.float32)
    for c in range(NCORES):
        b, h = divmod(c, 2)
        out[b][:, h * QH:(h + 1) * QH] = res[c]["out"]
    return out.reshape(B, C, H, W)


# revision 38
# speedup vs baseline: 1.0978x; 1.0098x over previous
"""CrossAttentionFusion Trainium2 kernel (nn_CrossAttentionFusion__45561013076033).

Full inputs -> full output. Sharding: 8 cores, core c handles batch b=c//2,
query-half h=c%2 (2048 of 4096 queries). Each core holds the full [256,4096]
cnn feature map of its batch (keys/values), its query-half of the transformer
features, and replicated weights.

v7 dataflow:
  - fp16 projections (Q/K/V written as fp8e4m3), fp16 fused conv
  - scores computed transposed (S^T tiles [key,query]) via fp8 DoubleRow
    matmuls (full 256-deep contraction per matmul); exp writes P^T fp8
    straight from PSUM
  - rowsums via fp8 DoubleRow ones-matmuls (replicated across partitions),
    fast-approx reciprocal, normalization folded into the attended output
  - scores / rowsum / attended matmuls interleaved per key-tile-pair within
    each 512-query superblock (consumers lag producers by one pair) so the
    PE never waits on the ACT exp stream
  - PE warm-up matmul chain at t~0 to open the HAM clock gate early
"""

import numpy as np

B, C, H, W = 4, 256, 64, 64
N = H * W            # 4096 tokens
NCORES = 8
QH = N // 2          # 2048 queries per core
CT = C // 128        # 2 channel tiles
KC = N // 512        # 8 key chunks of 512
NSB = QH // 512      # 4 query superblocks per core
NKT = N // 128       # 32 key tiles
NG = NKT // 2        # 16 key-tile pairs

_CACHE = {}


def _build():
    import concourse.bass as bass
    import concourse.mybir as mybir
    import concourse.tile as tile
    from concourse import bacc

    f32 = mybir.dt.float32
    f16 = mybir.dt.float16
    f8 = mybir.dt.float8e4
    DR = mybir.MatmulPerfMode.DoubleRow
    AF = mybir.ActivationFunctionType

    nc = bacc.Bacc("TRN2", target_bir_lowering=False, debug=True)

    WARM = nc.dram_tensor("warm", [128, 64], f32, kind="ExternalOutput")
    XQ = nc.dram_tensor("xq", [C, QH], f16, kind="ExternalInput")
    XC = nc.dram_tensor("xc", [C, N], f16, kind="ExternalInput")
    WQT = nc.dram_tensor("wqt", [C, C], f16, kind="ExternalInput")
    WKT = nc.dram_tensor("wkt", [C, C], f16, kind="ExternalInput")
    WVT = nc.dram_tensor("wvt", [C, C], f16, kind="ExternalInput")
    WFT = nc.dram_tensor("wft", [2 * C, C], f16, kind="ExternalInput")
    BQ = nc.dram_tensor("bq", [C], f32, kind="ExternalInput")
    BK = nc.dram_tensor("bk", [C], f32, kind="ExternalInput")
    BF = nc.dram_tensor("bf", [C], f32, kind="ExternalInput")
    OUT = nc.dram_tensor("out", [C, QH], f32, kind="ExternalOutput")

    xq_d = XQ.ap().rearrange("(t p) n -> p t n", p=128)
    xc_d = XC.ap().rearrange("(t p) n -> p t n", p=128)
    wq_d = WQT.ap().rearrange("(t p) d -> p t d", p=128)
    wk_d = WKT.ap().rearrange("(t p) d -> p t d", p=128)
    wv_d = WVT.ap().rearrange("(t p) d -> p t d", p=128)
    wf_d = WFT.ap().rearrange("(t p) d -> p t d", p=128)
    out_d = OUT.ap().rearrange("(t p) n -> p t n", p=128)

    with tile.TileContext(nc) as tc:
        with tc.tile_pool(name="persist", bufs=1) as per, \
             tc.tile_pool(name="ptp", bufs=2) as ptp, \
             tc.tile_pool(name="norm", bufs=2) as nrp, \
             tc.tile_pool(name="outp", bufs=2) as outp, \
             tc.tile_pool(name="sc", bufs=2, space="PSUM") as scp, \
             tc.tile_pool(name="rs", bufs=2, space="PSUM") as rsp, \
             tc.tile_pool(name="av", bufs=2, space="PSUM") as avp:

            # ---- persistent tiles ----
            xq_sb = per.tile([128, CT, QH], f16)
            xc_sb = per.tile([128, CT, N], f16)
            wq_sb = per.tile([128, CT, C], f16)
            wk_sb = per.tile([128, CT, C], f16)
            wv_sb = per.tile([128, CT, C], f16)
            wf_sb = per.tile([128, 2 * CT, C], f16)
            bq_sb = per.tile([128, CT], f32)
            bk_sb = per.tile([128, CT], f32)
            bf_sb = per.tile([128, CT], f32)
            q_sb = per.tile([128, CT, QH], f8)
            k_sb = per.tile([128, CT, N], f8)
            vt_sb = per.tile([128, NKT, C], f8)
            ones_sb = per.tile([128, 2, 128], f8)
            onesw_sb = per.tile([128, 128], f16)

            nc.sync.dma_start(wq_sb[:], wq_d)
            nc.sync.dma_start(wk_sb[:], wk_d)
            nc.sync.dma_start(wv_sb[:], wv_d)
            nc.sync.dma_start(wf_sb[:], wf_d)
            nc.sync.dma_start(bq_sb[:], BQ.ap().rearrange("(t p) -> p t", p=128))
            nc.sync.dma_start(bk_sb[:], BK.ap().rearrange("(t p) -> p t", p=128))
            nc.sync.dma_start(bf_sb[:], BF.ap().rearrange("(t p) -> p t", p=128))
            nc.vector.memset(ones_sb[:], 1.0)
            nc.vector.memset(onesw_sb[:], 1.0)

            # ---- PE warm-up: keep the PE busy from t~0 so the HAM clock
            # gate opens (1.2 -> 2.4 GHz) before the real matmuls start.
            wps = scp.tile([128, 512], f32, tag="sc")
            for _ in range(120):
                nc.tensor.matmul(wps[:, :64], onesw_sb[:], onesw_sb[:, :64],
                                 start=True, stop=True)
            w_sb = per.tile([128, 64], f32)
            nc.scalar.activation(w_sb[:], wps[:, :64], AF.Copy)
            nc.sync.dma_start(WARM.ap(), w_sb[:])

            # ---- inputs + projections, interleaved per 512-column chunk ----
            # Q projection: Q[d, n], bias added on DVE
            for qc in range(QH // 512):
                s = slice(qc * 512, (qc + 1) * 512)
                for ct in range(CT):
                    nc.sync.dma_start(xq_sb[:, ct, s], xq_d[:, ct, s])
                for dt in range(CT):
                    ps = scp.tile([128, 512], f32, tag="sc")
                    for ct in range(CT):
                        nc.tensor.matmul(
                            ps[:], wq_sb[:, ct, dt * 128:(dt + 1) * 128],
                            xq_sb[:, ct, s],
                            start=(ct == 0), stop=(ct == CT - 1))
                    nc.vector.tensor_scalar_add(
                        q_sb[:, dt, s], ps[:], bq_sb[:, dt:dt + 1])

            # K + V^T projections per xc chunk
            for kc in range(KC):
                s = slice(kc * 512, (kc + 1) * 512)
                for ct in range(CT):
                    nc.sync.dma_start(xc_sb[:, ct, s], xc_d[:, ct, s])
                for dt in range(CT):
                    ps = scp.tile([128, 512], f32, tag="sc")
                    for ct in range(CT):
                        nc.tensor.matmul(
                            ps[:], wk_sb[:, ct, dt * 128:(dt + 1) * 128],
                            xc_sb[:, ct, s],
                            start=(ct == 0), stop=(ct == CT - 1))
                    nc.vector.tensor_scalar_add(
                        k_sb[:, dt, s], ps[:], bk_sb[:, dt:dt + 1])
                # V^T: [keys, d] (no bias; folded into conv bias)
                for j in range(4):
                    kt = 4 * kc + j
                    ps = scp.tile([128, 512], f32, tag="sc")
                    for ct in range(CT):
                        nc.tensor.matmul(
                            ps[:, :C], xc_sb[:, ct, kt * 128:(kt + 1) * 128],
                            wv_sb[:, ct],
                            start=(ct == 0), stop=(ct == CT - 1))
                    nc.vector.tensor_copy(vt_sb[:, kt], ps[:, :C])

            # ---- attention + fused conv, per 512-query superblock ----
            # Per key-tile pair g: two fp8-DR score matmuls + exps feed the
            # rowsum + attended matmuls of pair g-1 (one-pair lag keeps the
            # PE from waiting on ACT). Previous superblock's normalize/conv
            # are emitted after this superblock's first pairs.
            nc._v7_tiles = (outp, rsp, wf_sb, xq_sb, bf_sb, out_d)
            prev = None
            for sb in range(NSB):
                qs = slice(sb * 512, (sb + 1) * 512)
                pt = ptp.tile([128, NKT, 512], f8, tag="pt")
                rs = rsp.tile([128, 512], f32, tag="rs")
                aps = avp.tile([128, 2, 512], f32, tag="av")

                def scores_pair(g):
                    for j in range(2):
                        kt = 2 * g + j
                        ps = scp.tile([128, 512], f32, tag="sc")
                        nc.tensor.matmul(
                            ps[:], k_sb[:, :, kt * 128:(kt + 1) * 128],
                            q_sb[:, :, qs],
                            start=True, stop=True, perf_mode=DR)
                        nc.scalar.activation(pt[:, kt], ps[:], AF.Exp)

                def consume_pair(g):
                    nc.tensor.matmul(
                        rs[:], ones_sb[:], pt[:, 2 * g:2 * (g + 1)],
                        start=(g == 0), stop=(g == NG - 1), perf_mode=DR)
                    for dt in range(CT):
                        nc.tensor.matmul(
                            aps[:, dt],
                            vt_sb[:, 2 * g:2 * (g + 1),
                                  dt * 128:(dt + 1) * 128],
                            pt[:, 2 * g:2 * (g + 1)],
                            start=(g == 0), stop=(g == NG - 1),
                            perf_mode=DR)

                scores_pair(0)
                scores_pair(1)
                if prev is not None:
                    _finish_sb(nc, mybir, prev)
                    prev = None
                for g in range(2, NG):
                    scores_pair(g)
                    consume_pair(g - 2)
                consume_pair(NG - 2)
                consume_pair(NG - 1)

                rinv = nrp.tile([128, 512], f32, tag="rinv")
                nc.vector.reciprocal_approx_fast(rinv[:], rs[:])
                prev = dict(qs=qs, aps=aps, rinv=rinv)

            _finish_sb(nc, mybir, prev)
    nc.finalize()
    return nc


def _finish_sb(nc, mybir, env):
    """Normalize attended output, fused 1x1 conv, bias, DMA out."""
    f32 = mybir.dt.float32
    f16 = mybir.dt.float16
    tiles = nc._v7_tiles
    qs, aps, rinv = env["qs"], env["aps"], env["rinv"]
    (outp, rsp, wf_sb, xq_sb, bf_sb, out_d) = tiles

    a_sb = outp.tile([128, CT, 512], f16, tag="a")
    for dt in range(CT):
        nc.vector.tensor_mul(a_sb[:, dt], aps[:, dt], rinv[:])
    o_sb = outp.tile([128, CT, 512], f32, tag="o")
    for dt in range(CT):
        ops = rsp.tile([128, 512], f32, tag="rs")
        for j in range(2 * CT):
            rhs = xq_sb[:, j, qs] if j < CT else a_sb[:, j - CT]
            nc.tensor.matmul(
                ops[:], wf_sb[:, j, dt * 128:(dt + 1) * 128],
                rhs, start=(j == 0), stop=(j == 2 * CT - 1))
        nc.vector.tensor_scalar_add(
            o_sb[:, dt], ops[:], bf_sb[:, dt:dt + 1])
        nc.sync.dma_start(out_d[:, dt, qs], o_sb[:, dt])


def _get_nc():
    if "nc" not in _CACHE:
        _CACHE["nc"] = _build()
    return _CACHE["nc"]


def _in_maps(transformer_features, cnn_features, Wq, bq, Wk, bk, Wv, bv, Wf, bf):
    xt = np.ascontiguousarray(np.asarray(transformer_features, np.float32)
                              .reshape(B, C, N))
    xc = np.ascontiguousarray(np.asarray(cnn_features, np.float32)
                              .reshape(B, C, N))
    Wq = np.asarray(Wq, np.float32)
    Wk = np.asarray(Wk, np.float32)
    Wv = np.asarray(Wv, np.float32)
    Wf = np.asarray(Wf, np.float32)
    bq = np.asarray(bq, np.float32)
    bk = np.asarray(bk, np.float32)
    bv = np.asarray(bv, np.float32)
    bf = np.asarray(bf, np.float32)

    wqt = np.ascontiguousarray(Wq.T / 16.0).astype(np.float16)
    wkt = np.ascontiguousarray(Wk.T).astype(np.float16)
    wvt = np.ascontiguousarray(Wv.T).astype(np.float16)
    wft = np.ascontiguousarray(Wf.T).astype(np.float16)
    bq_s = bq / 16.0
    bf2 = bf + Wf[:, C:] @ bv
    xt16 = xt.astype(np.float16)
    xc16 = xc.astype(np.float16)

    maps = []
    for c in range(NCORES):
        b, h = divmod(c, 2)
        maps.append(dict(
            xq=np.ascontiguousarray(xt16[b][:, h * QH:(h + 1) * QH]),
            xc=xc16[b],
            wqt=wqt, wkt=wkt, wvt=wvt, wft=wft,
            bq=bq_s, bk=bk, bf=bf2,
        ))
    return maps


def _run(inputs, trace=False):
    from concourse.bass_utils import run_bass_kernel_spmd
    nc = _get_nc()
    maps = _in_maps(**inputs)
    return run_bass_kernel_spmd(nc, maps, list(range(NCORES)), trace=trace)


def kernel(**inputs) -> np.ndarray:
    res = _run(inputs).results
    out = np.empty((B, C, N), np.float32)
    for c in range(NCORES):
        b, h = divmod(c, 2)
        out[b][:, h * QH:(h + 1) * QH] = res[c]["out"]
    return out.reshape(B, C, H, W)


# revision 42
# speedup vs baseline: 1.1720x; 1.0677x over previous
"""CrossAttentionFusion Trainium2 kernel (nn_CrossAttentionFusion__45561013076033).

Full inputs -> full output. Sharding: 8 cores, core c handles batch b=c//2,
query-half h=c%2 (2048 of 4096 queries). Each core holds the full [256,4096]
cnn feature map of its batch (keys/values), its query-half of the transformer
features, and replicated weights.

v7 dataflow:
  - fp16 projections (Q/K/V written as fp8e4m3), fp16 fused conv
  - scores computed transposed (S^T tiles [key,query]) via fp8 DoubleRow
    matmuls (full 256-deep contraction per matmul); exp writes P^T fp8
    straight from PSUM
  - rowsums via fp8 DoubleRow ones-matmuls (replicated across partitions),
    fast-approx reciprocal, normalization folded into the attended output
  - scores / rowsum / attended matmuls interleaved per key-tile-pair within
    each 512-query superblock (consumers lag producers by one pair) so the
    PE never waits on the ACT exp stream
  - PE warm-up matmul chain at t~0 to open the HAM clock gate early
"""

import numpy as np

B, C, H, W = 4, 256, 64, 64
N = H * W            # 4096 tokens
NCORES = 8
QH = N // 2          # 2048 queries per core
CT = C // 128        # 2 channel tiles
KC = N // 512        # 8 key chunks of 512
NSB = QH // 512      # 4 query superblocks per core
NKT = N // 128       # 32 key tiles
NG = NKT // 2        # 16 key-tile pairs

_CACHE = {}


def _build():
    import concourse.bass as bass
    import concourse.mybir as mybir
    import concourse.tile as tile
    from concourse import bacc

    f32 = mybir.dt.float32
    f16 = mybir.dt.float16
    f8 = mybir.dt.float8e4
    DR = mybir.MatmulPerfMode.DoubleRow
    AF = mybir.ActivationFunctionType

    nc = bacc.Bacc("TRN2", target_bir_lowering=False, debug=True)

    WARM = nc.dram_tensor("warm", [128, 64], f32, kind="ExternalOutput")
    XQ = nc.dram_tensor("xq", [C, QH], f16, kind="ExternalInput")
    XC = nc.dram_tensor("xc", [C, N], f16, kind="ExternalInput")
    WQT = nc.dram_tensor("wqt", [C, C], f16, kind="ExternalInput")
    WKT = nc.dram_tensor("wkt", [C, C], f16, kind="ExternalInput")
    WVT = nc.dram_tensor("wvt", [C, C], f16, kind="ExternalInput")
    WFT = nc.dram_tensor("wft", [2 * C, C], f16, kind="ExternalInput")
    BQ = nc.dram_tensor("bq", [C], f32, kind="ExternalInput")
    BK = nc.dram_tensor("bk", [C], f32, kind="ExternalInput")
    BF = nc.dram_tensor("bf", [C], f32, kind="ExternalInput")
    OUT = nc.dram_tensor("out", [C, QH], f32, kind="ExternalOutput")

    xq_d = XQ.ap().rearrange("(t p) n -> p t n", p=128)
    xc_d = XC.ap().rearrange("(t p) n -> p t n", p=128)
    wq_d = WQT.ap().rearrange("(t p) d -> p t d", p=128)
    wk_d = WKT.ap().rearrange("(t p) d -> p t d", p=128)
    wv_d = WVT.ap().rearrange("(t p) d -> p t d", p=128)
    wf_d = WFT.ap().rearrange("(t p) d -> p t d", p=128)
    out_d = OUT.ap().rearrange("(t p) n -> p t n", p=128)

    with tile.TileContext(nc) as tc:
        with tc.tile_pool(name="persist", bufs=1) as per, \
             tc.tile_pool(name="ptp", bufs=2) as ptp, \
             tc.tile_pool(name="norm", bufs=2) as nrp, \
             tc.tile_pool(name="outp", bufs=2) as outp, \
             tc.tile_pool(name="sc", bufs=3, space="PSUM") as scp, \
             tc.tile_pool(name="rs", bufs=1, space="PSUM") as rsp, \
             tc.tile_pool(name="av", bufs=2, space="PSUM") as avp:

            # ---- persistent tiles ----
            xq_sb = per.tile([128, CT, QH], f16)
            xc_sb = per.tile([128, CT, N], f16)
            wq_sb = per.tile([128, CT, C], f16)
            wk_sb = per.tile([128, CT, C], f16)
            wv_sb = per.tile([128, CT, C], f16)
            wf_sb = per.tile([128, 2 * CT, C], f16)
            bq_sb = per.tile([128, CT], f32)
            bk_sb = per.tile([128, CT], f32)
            bf_sb = per.tile([128, CT], f32)
            q_sb = per.tile([128, CT, QH], f8)
            k_sb = per.tile([128, CT, N], f8)
            vt_sb = per.tile([128, NKT, C], f8)
            ones_sb = per.tile([128, 2, 128], f8)
            onesw_sb = per.tile([128, 128], f16)

            nc.sync.dma_start(wq_sb[:], wq_d)
            nc.sync.dma_start(wk_sb[:], wk_d)
            nc.sync.dma_start(wv_sb[:], wv_d)
            nc.sync.dma_start(wf_sb[:], wf_d)
            nc.sync.dma_start(bq_sb[:], BQ.ap().rearrange("(t p) -> p t", p=128))
            nc.sync.dma_start(bk_sb[:], BK.ap().rearrange("(t p) -> p t", p=128))
            nc.sync.dma_start(bf_sb[:], BF.ap().rearrange("(t p) -> p t", p=128))
            nc.vector.memset(ones_sb[:], 1.0)
            nc.vector.memset(onesw_sb[:], 1.0)

            # ---- PE warm-up: keep the PE busy from t~0 so the HAM clock
            # gate opens (1.2 -> 2.4 GHz) before the real matmuls start.
            wps = scp.tile([128, 512], f32, tag="sc")
            for _ in range(120):
                nc.tensor.matmul(wps[:, :64], onesw_sb[:], onesw_sb[:, :64],
                                 start=True, stop=True)
            w_sb = per.tile([128, 64], f32)
            nc.scalar.activation(w_sb[:], wps[:, :64], AF.Copy)
            nc.sync.dma_start(WARM.ap(), w_sb[:])

            # ---- inputs + projections, interleaved per 512-column chunk ----
            # Q projection: Q[d, n], bias added on DVE
            for qc in range(QH // 512):
                s = slice(qc * 512, (qc + 1) * 512)
                for ct in range(CT):
                    nc.sync.dma_start(xq_sb[:, ct, s], xq_d[:, ct, s])
                for dt in range(CT):
                    ps = scp.tile([128, 512], f32, tag="sc")
                    for ct in range(CT):
                        nc.tensor.matmul(
                            ps[:], wq_sb[:, ct, dt * 128:(dt + 1) * 128],
                            xq_sb[:, ct, s],
                            start=(ct == 0), stop=(ct == CT - 1))
                    nc.vector.tensor_scalar_add(
                        q_sb[:, dt, s], ps[:], bq_sb[:, dt:dt + 1])

            # K + V^T projections per xc chunk
            for kc in range(KC):
                s = slice(kc * 512, (kc + 1) * 512)
                for ct in range(CT):
                    nc.sync.dma_start(xc_sb[:, ct, s], xc_d[:, ct, s])
                for dt in range(CT):
                    ps = scp.tile([128, 512], f32, tag="sc")
                    for ct in range(CT):
                        nc.tensor.matmul(
                            ps[:], wk_sb[:, ct, dt * 128:(dt + 1) * 128],
                            xc_sb[:, ct, s],
                            start=(ct == 0), stop=(ct == CT - 1))
                    nc.vector.tensor_scalar_add(
                        k_sb[:, dt, s], ps[:], bk_sb[:, dt:dt + 1])
                # V^T: [keys, d] (no bias; folded into conv bias)
                for j in range(4):
                    kt = 4 * kc + j
                    ps = scp.tile([128, 512], f32, tag="sc")
                    for ct in range(CT):
                        nc.tensor.matmul(
                            ps[:, :C], xc_sb[:, ct, kt * 128:(kt + 1) * 128],
                            wv_sb[:, ct],
                            start=(ct == 0), stop=(ct == CT - 1))
                    nc.vector.tensor_copy(vt_sb[:, kt], ps[:, :C])

            # ---- attention + fused conv, per 512-query superblock ----
            # Per key-tile pair g: two fp8-DR score matmuls + exps feed the
            # rowsum + attended matmuls of pair g-1 (one-pair lag keeps the
            # PE from waiting on ACT). Previous superblock's normalize/conv
            # are emitted after this superblock's first pairs.
            nc._v7_tiles = (outp, scp, wf_sb, xq_sb, bf_sb, out_d)
            prev = None
            for sb in range(NSB):
                qs = slice(sb * 512, (sb + 1) * 512)
                pt = ptp.tile([128, NKT, 512], f8, tag="pt")
                rs = rsp.tile([128, 512], f32, tag="rs")
                aps = avp.tile([128, 2, 512], f32, tag="av")

                def scores_pair(g):
                    for j in range(2):
                        kt = 2 * g + j
                        ps = scp.tile([128, 512], f32, tag="sc")
                        nc.tensor.matmul(
                            ps[:], k_sb[:, :, kt * 128:(kt + 1) * 128],
                            q_sb[:, :, qs],
                            start=True, stop=True, perf_mode=DR)
                        nc.scalar.activation(pt[:, kt], ps[:], AF.Exp)

                def consume_pair(g):
                    nc.tensor.matmul(
                        rs[:], ones_sb[:], pt[:, 2 * g:2 * (g + 1)],
                        start=(g == 0), stop=(g == NG - 1), perf_mode=DR)
                    for dt in range(CT):
                        nc.tensor.matmul(
                            aps[:, dt],
                            vt_sb[:, 2 * g:2 * (g + 1),
                                  dt * 128:(dt + 1) * 128],
                            pt[:, 2 * g:2 * (g + 1)],
                            start=(g == 0), stop=(g == NG - 1),
                            perf_mode=DR)

                scores_pair(0)
                scores_pair(1)
                if prev is not None:
                    _finish_sb(nc, mybir, prev)
                    prev = None
                for g in range(2, NG):
                    scores_pair(g)
                    consume_pair(g - 2)
                consume_pair(NG - 2)
                consume_pair(NG - 1)

                rinv = nrp.tile([128, 512], f32, tag="rinv")
                nc.vector.reciprocal_approx_fast(rinv[:], rs[:])
                prev = dict(qs=qs, aps=aps, rinv=rinv)

            _finish_sb(nc, mybir, prev)
    nc.finalize()
    return nc


def _finish_sb(nc, mybir, env):
    """Normalize attended output, fused 1x1 conv, bias, DMA out."""
    f32 = mybir.dt.float32
    f16 = mybir.dt.float16
    tiles = nc._v7_tiles
    qs, aps, rinv = env["qs"], env["aps"], env["rinv"]
    (outp, scp, wf_sb, xq_sb, bf_sb, out_d) = tiles

    a_sb = outp.tile([128, CT, 512], f16, tag="a")
    for dt in range(CT):
        nc.vector.tensor_mul(a_sb[:, dt], aps[:, dt], rinv[:])
    o_sb = outp.tile([128, CT, 512], f32, tag="o")
    for dt in range(CT):
        ops = scp.tile([128, 512], f32, tag="sc")
        for j in range(2 * CT):
            rhs = xq_sb[:, j, qs] if j < CT else a_sb[:, j - CT]
            nc.tensor.matmul(
                ops[:], wf_sb[:, j, dt * 128:(dt + 1) * 128],
                rhs, start=(j == 0), stop=(j == 2 * CT - 1))
        nc.vector.tensor_scalar_add(
            o_sb[:, dt], ops[:], bf_sb[:, dt:dt + 1])
        nc.sync.dma_start(out_d[:, dt, qs], o_sb[:, dt])


def _get_nc():
    if "nc" not in _CACHE:
        _CACHE["nc"] = _build()
    return _CACHE["nc"]


def _in_maps(transformer_features, cnn_features, Wq, bq, Wk, bk, Wv, bv, Wf, bf):
    xt = np.ascontiguousarray(np.asarray(transformer_features, np.float32)
                              .reshape(B, C, N))
    xc = np.ascontiguousarray(np.asarray(cnn_features, np.float32)
                              .reshape(B, C, N))
    Wq = np.asarray(Wq, np.float32)
    Wk = np.asarray(Wk, np.float32)
    Wv = np.asarray(Wv, np.float32)
    Wf = np.asarray(Wf, np.float32)
    bq = np.asarray(bq, np.float32)
    bk = np.asarray(bk, np.float32)
    bv = np.asarray(bv, np.float32)
    bf = np.asarray(bf, np.float32)

    wqt = np.ascontiguousarray(Wq.T / 16.0).astype(np.float16)
    wkt = np.ascontiguousarray(Wk.T).astype(np.float16)
    wvt = np.ascontiguousarray(Wv.T).astype(np.float16)
    wft = np.ascontiguousarray(Wf.T).astype(np.float16)
    bq_s = bq / 16.0
    bf2 = bf + Wf[:, C:] @ bv
    xt16 = xt.astype(np.float16)
    xc16 = xc.astype(np.float16)

    maps = []
    for c in range(NCORES):
        b, h = divmod(c, 2)
        maps.append(dict(
            xq=np.ascontiguousarray(xt16[b][:, h * QH:(h + 1) * QH]),
            xc=xc16[b],
            wqt=wqt, wkt=wkt, wvt=wvt, wft=wft,
            bq=bq_s, bk=bk, bf=bf2,
        ))
    return maps


def _run(inputs, trace=False):
    from concourse.bass_utils import run_bass_kernel_spmd
    nc = _get_nc()
    maps = _in_maps(**inputs)
    return run_bass_kernel_spmd(nc, maps, list(range(NCORES)), trace=trace)


def kernel(**inputs) -> np.ndarray:
    res = _run(inputs).results
    out = np.empty((B, C, N), np.float32)
    for c in range(NCORES):
        b, h = divmod(c, 2)
        out[b][:, h * QH:(h + 1) * QH] = res[c]["out"]
    return out.reshape(B, C, H, W)


# revision 45
# speedup vs baseline: 1.1893x; 1.0147x over previous
"""CrossAttentionFusion Trainium2 kernel (nn_CrossAttentionFusion__45561013076033).

Full inputs -> full output. Sharding: 8 cores, core c handles batch b=c//2,
query-half h=c%2 (2048 of 4096 queries). Each core holds the full [256,4096]
cnn feature map of its batch (keys/values), its query-half of the transformer
features, and replicated weights.

v7 dataflow:
  - fp16 projections (Q/K/V written as fp8e4m3), fp16 fused conv
  - scores computed transposed (S^T tiles [key,query]) via fp8 DoubleRow
    matmuls (full 256-deep contraction per matmul); exp writes P^T fp8
    straight from PSUM
  - rowsums via fp8 DoubleRow ones-matmuls (replicated across partitions),
    fast-approx reciprocal, normalization folded into the attended output
  - scores / rowsum / attended matmuls interleaved per key-tile-pair within
    each 512-query superblock (consumers lag producers by one pair) so the
    PE never waits on the ACT exp stream
  - PE warm-up matmul chain at t~0 to open the HAM clock gate early
"""

import numpy as np

B, C, H, W = 4, 256, 64, 64
N = H * W            # 4096 tokens
NCORES = 8
QH = N // 2          # 2048 queries per core
CT = C // 128        # 2 channel tiles
KC = N // 512        # 8 key chunks of 512
NSB = QH // 512      # 4 query superblocks per core
NKT = N // 128       # 32 key tiles
NG = NKT // 2        # 16 key-tile pairs

_CACHE = {}


def _build():
    import concourse.bass as bass
    import concourse.mybir as mybir
    import concourse.tile as tile
    from concourse import bacc

    f32 = mybir.dt.float32
    f16 = mybir.dt.float16
    f8 = mybir.dt.float8e4
    DR = mybir.MatmulPerfMode.DoubleRow
    AF = mybir.ActivationFunctionType

    nc = bacc.Bacc("TRN2", target_bir_lowering=False, debug=True)

    WARM = nc.dram_tensor("warm", [128, 64], f32, kind="ExternalOutput")
    XQ = nc.dram_tensor("xq", [C, QH], f16, kind="ExternalInput")
    XC = nc.dram_tensor("xc", [C, N], f16, kind="ExternalInput")
    WQT = nc.dram_tensor("wqt", [C, C], f16, kind="ExternalInput")
    WKT = nc.dram_tensor("wkt", [C, C], f16, kind="ExternalInput")
    WVT = nc.dram_tensor("wvt", [C, C], f16, kind="ExternalInput")
    WFT = nc.dram_tensor("wft", [2 * C, C], f16, kind="ExternalInput")
    BQ = nc.dram_tensor("bq", [C], f32, kind="ExternalInput")
    BK = nc.dram_tensor("bk", [C], f32, kind="ExternalInput")
    BF = nc.dram_tensor("bf", [C], f32, kind="ExternalInput")
    OUT = nc.dram_tensor("out", [C, QH], f32, kind="ExternalOutput")

    xq_d = XQ.ap().rearrange("(t p) n -> p t n", p=128)
    xc_d = XC.ap().rearrange("(t p) n -> p t n", p=128)
    wq_d = WQT.ap().rearrange("(t p) d -> p t d", p=128)
    wk_d = WKT.ap().rearrange("(t p) d -> p t d", p=128)
    wv_d = WVT.ap().rearrange("(t p) d -> p t d", p=128)
    wf_d = WFT.ap().rearrange("(t p) d -> p t d", p=128)
    out_d = OUT.ap().rearrange("(t p) n -> p t n", p=128)

    with tile.TileContext(nc) as tc:
        with tc.tile_pool(name="persist", bufs=1) as per, \
             tc.tile_pool(name="ptp", bufs=2) as ptp, \
             tc.tile_pool(name="norm", bufs=2) as nrp, \
             tc.tile_pool(name="outp", bufs=2) as outp, \
             tc.tile_pool(name="sc", bufs=3, space="PSUM") as scp, \
             tc.tile_pool(name="rs", bufs=1, space="PSUM") as rsp, \
             tc.tile_pool(name="av", bufs=2, space="PSUM") as avp:

            # ---- persistent tiles ----
            xq_sb = per.tile([128, CT, QH], f16)
            xc_sb = per.tile([128, CT, N], f16)
            wq_sb = per.tile([128, CT, C], f16)
            wk_sb = per.tile([128, CT, C], f16)
            wv_sb = per.tile([128, CT, C], f16)
            wf_sb = per.tile([128, 2 * CT, C], f16)
            bq_sb = per.tile([128, CT], f32)
            bk_sb = per.tile([128, CT], f32)
            bf_sb = per.tile([128, CT], f32)
            q_sb = per.tile([128, CT, QH], f8)
            k_sb = per.tile([128, CT, N], f8)
            vt_sb = per.tile([128, NKT, C], f8)
            ones_sb = per.tile([128, 2, 128], f8)
            onesw_sb = per.tile([128, 128], f16)

            nc.sync.dma_start(wq_sb[:], wq_d)
            nc.sync.dma_start(wk_sb[:], wk_d)
            nc.sync.dma_start(wv_sb[:], wv_d)
            nc.sync.dma_start(wf_sb[:], wf_d)
            nc.sync.dma_start(bq_sb[:], BQ.ap().rearrange("(t p) -> p t", p=128))
            nc.sync.dma_start(bk_sb[:], BK.ap().rearrange("(t p) -> p t", p=128))
            nc.sync.dma_start(bf_sb[:], BF.ap().rearrange("(t p) -> p t", p=128))
            nc.vector.memset(ones_sb[:], 1.0)
            nc.vector.memset(onesw_sb[:], 1.0)

            # ---- PE warm-up: keep the PE busy from t~0 so the HAM clock
            # gate opens (1.2 -> 2.4 GHz) before the real matmuls start.
            wps = scp.tile([128, 512], f32, tag="sc")
            for _ in range(90):
                nc.tensor.matmul(wps[:, :64], onesw_sb[:], onesw_sb[:, :64],
                                 start=True, stop=True)
            w_sb = per.tile([128, 64], f32)
            nc.scalar.activation(w_sb[:], wps[:, :64], AF.Copy)
            nc.sync.dma_start(WARM.ap(), w_sb[:])

            # ---- inputs + projections, interleaved per 512-column chunk ----
            # Q projection: Q[d, n], bias added on DVE
            for qc in range(QH // 512):
                s = slice(qc * 512, (qc + 1) * 512)
                for ct in range(CT):
                    nc.sync.dma_start(xq_sb[:, ct, s], xq_d[:, ct, s])
                for dt in range(CT):
                    ps = scp.tile([128, 512], f32, tag="sc")
                    for ct in range(CT):
                        nc.tensor.matmul(
                            ps[:], wq_sb[:, ct, dt * 128:(dt + 1) * 128],
                            xq_sb[:, ct, s],
                            start=(ct == 0), stop=(ct == CT - 1))
                    nc.vector.tensor_scalar_add(
                        q_sb[:, dt, s], ps[:], bq_sb[:, dt:dt + 1])

            # K + V^T projections per xc chunk
            for kc in range(KC):
                s = slice(kc * 512, (kc + 1) * 512)
                for ct in range(CT):
                    nc.sync.dma_start(xc_sb[:, ct, s], xc_d[:, ct, s])
                for dt in range(CT):
                    ps = scp.tile([128, 512], f32, tag="sc")
                    for ct in range(CT):
                        nc.tensor.matmul(
                            ps[:], wk_sb[:, ct, dt * 128:(dt + 1) * 128],
                            xc_sb[:, ct, s],
                            start=(ct == 0), stop=(ct == CT - 1))
                    nc.vector.tensor_scalar_add(
                        k_sb[:, dt, s], ps[:], bk_sb[:, dt:dt + 1])
                # V^T: [keys, d] (no bias; folded into conv bias)
                for j in range(4):
                    kt = 4 * kc + j
                    ps = scp.tile([128, 512], f32, tag="sc")
                    for ct in range(CT):
                        nc.tensor.matmul(
                            ps[:, :C], xc_sb[:, ct, kt * 128:(kt + 1) * 128],
                            wv_sb[:, ct],
                            start=(ct == 0), stop=(ct == CT - 1))
                    nc.vector.tensor_copy(vt_sb[:, kt], ps[:, :C])

            # ---- attention + fused conv, per 512-query superblock ----
            # Per key-tile pair g: two fp8-DR score matmuls + exps feed the
            # rowsum + attended matmuls of pair g-1 (one-pair lag keeps the
            # PE from waiting on ACT). Previous superblock's normalize/conv
            # are emitted after this superblock's first pairs.
            nc._v7_tiles = (outp, scp, wf_sb, xq_sb, bf_sb, out_d)
            prev = None
            for sb in range(NSB):
                qs = slice(sb * 512, (sb + 1) * 512)
                pt = ptp.tile([128, NKT, 512], f8, tag="pt")
                rs = rsp.tile([128, 512], f32, tag="rs")
                aps = avp.tile([128, 2, 512], f32, tag="av")

                def scores_pair(g):
                    for j in range(2):
                        kt = 2 * g + j
                        ps = scp.tile([128, 512], f32, tag="sc")
                        nc.tensor.matmul(
                            ps[:], k_sb[:, :, kt * 128:(kt + 1) * 128],
                            q_sb[:, :, qs],
                            start=True, stop=True, perf_mode=DR)
                        nc.scalar.activation(pt[:, kt], ps[:], AF.Exp)

                def consume_pair(g):
                    nc.tensor.matmul(
                        rs[:], ones_sb[:], pt[:, 2 * g:2 * (g + 1)],
                        start=(g == 0), stop=(g == NG - 1), perf_mode=DR)
                    for dt in range(CT):
                        nc.tensor.matmul(
                            aps[:, dt],
                            vt_sb[:, 2 * g:2 * (g + 1),
                                  dt * 128:(dt + 1) * 128],
                            pt[:, 2 * g:2 * (g + 1)],
                            start=(g == 0), stop=(g == NG - 1),
                            perf_mode=DR)

                scores_pair(0)
                scores_pair(1)
                if prev is not None:
                    _finish_sb(nc, mybir, prev)
                    prev = None
                for g in range(2, NG):
                    scores_pair(g)
                    consume_pair(g - 2)
                consume_pair(NG - 2)
                consume_pair(NG - 1)

                rinv = nrp.tile([128, 512], f32, tag="rinv")
                if sb < NSB - 1:
                    nc.vector.reciprocal_approx_fast(rinv[:], rs[:])
                else:
                    # split final reciprocal so the tail chain starts earlier
                    nc.vector.reciprocal_approx_fast(rinv[:, :256],
                                                     rs[:, :256])
                    nc.vector.reciprocal_approx_fast(rinv[:, 256:],
                                                     rs[:, 256:])
                prev = dict(qs=qs, aps=aps, rinv=rinv)

            _finish_sb(nc, mybir, prev, halves=True)
    nc.finalize()
    return nc


def _finish_sb(nc, mybir, env, halves=False):
    """Normalize attended output, fused 1x1 conv, bias, DMA out."""
    f32 = mybir.dt.float32
    f16 = mybir.dt.float16
    tiles = nc._v7_tiles
    qs, aps, rinv = env["qs"], env["aps"], env["rinv"]
    (outp, scp, wf_sb, xq_sb, bf_sb, out_d) = tiles

    a_sb = outp.tile([128, CT, 512], f16, tag="a")
    o_sb = outp.tile([128, CT, 512], f32, tag="o")
    hs = ((slice(0, 256), slice(256, 512)) if halves
          else (slice(0, 512),))
    for h in hs:
        hq = slice(qs.start + h.start, qs.start + h.stop)
        for dt in range(CT):
            nc.vector.tensor_mul(a_sb[:, dt, h], aps[:, dt, h], rinv[:, h])
        for dt in range(CT):
            ops = scp.tile([128, 512], f32, tag="sc")
            for j in range(2 * CT):
                rhs = (xq_sb[:, j, hq] if j < CT
                       else a_sb[:, j - CT, h])
                nc.tensor.matmul(
                    ops[:, h], wf_sb[:, j, dt * 128:(dt + 1) * 128],
                    rhs, start=(j == 0), stop=(j == 2 * CT - 1))
            nc.vector.tensor_scalar_add(
                o_sb[:, dt, h], ops[:, h], bf_sb[:, dt:dt + 1])
            nc.sync.dma_start(out_d[:, dt, hq], o_sb[:, dt, h])


def _get_nc():
    if "nc" not in _CACHE:
        _CACHE["nc"] = _build()
    return _CACHE["nc"]


def _in_maps(transformer_features, cnn_features, Wq, bq, Wk, bk, Wv, bv, Wf, bf):
    xt = np.ascontiguousarray(np.asarray(transformer_features, np.float32)
                              .reshape(B, C, N))
    xc = np.ascontiguousarray(np.asarray(cnn_features, np.float32)
                              .reshape(B, C, N))
    Wq = np.asarray(Wq, np.float32)
    Wk = np.asarray(Wk, np.float32)
    Wv = np.asarray(Wv, np.float32)
    Wf = np.asarray(Wf, np.float32)
    bq = np.asarray(bq, np.float32)
    bk = np.asarray(bk, np.float32)
    bv = np.asarray(bv, np.float32)
    bf = np.asarray(bf, np.float32)

    wqt = np.ascontiguousarray(Wq.T / 16.0).astype(np.float16)
    wkt = np.ascontiguousarray(Wk.T).astype(np.float16)
    wvt = np.ascontiguousarray(Wv.T).astype(np.float16)
    wft = np.ascontiguousarray(Wf.T).astype(np.float16)
    bq_s = bq / 16.0
    bf2 = bf + Wf[:, C:] @ bv
    xt16 = xt.astype(np.float16)
    xc16 = xc.astype(np.float16)

    maps = []
    for c in range(NCORES):
        b, h = divmod(c, 2)
        maps.append(dict(
            xq=np.ascontiguousarray(xt16[b][:, h * QH:(h + 1) * QH]),
            xc=xc16[b],
            wqt=wqt, wkt=wkt, wvt=wvt, wft=wft,
            bq=bq_s, bk=bk, bf=bf2,
        ))
    return maps


def _run(inputs, trace=False):
    from concourse.bass_utils import run_bass_kernel_spmd
    nc = _get_nc()
    maps = _in_maps(**inputs)
    return run_bass_kernel_spmd(nc, maps, list(range(NCORES)), trace=trace)


def kernel(**inputs) -> np.ndarray:
    res = _run(inputs).results
    out = np.empty((B, C, N), np.float32)
    for c in range(NCORES):
        b, h = divmod(c, 2)
        out[b][:, h * QH:(h + 1) * QH] = res[c]["out"]
    return out.reshape(B, C, H, W)


# revision 49
# speedup vs baseline: 1.2218x; 1.0273x over previous
"""CrossAttentionFusion Trainium2 kernel (nn_CrossAttentionFusion__45561013076033).

Full inputs -> full output. Sharding: 8 cores, core c handles batch b=c//2,
query-half h=c%2 (2048 of 4096 queries). Each core holds the full [256,4096]
cnn feature map of its batch (keys/values), its query-half of the transformer
features, and replicated weights.

v7 dataflow:
  - fp16 projections (Q/K/V written as fp8e4m3), fp16 fused conv
  - scores computed transposed (S^T tiles [key,query]) via fp8 DoubleRow
    matmuls (full 256-deep contraction per matmul); exp writes P^T fp8
    straight from PSUM
  - rowsums via fp8 DoubleRow ones-matmuls (replicated across partitions),
    fast-approx reciprocal, normalization folded into the attended output
  - scores / rowsum / attended matmuls interleaved per key-tile-pair within
    each 512-query superblock (consumers lag producers by one pair) so the
    PE never waits on the ACT exp stream
  - PE warm-up matmul chain at t~0 to open the HAM clock gate early
"""

import numpy as np

B, C, H, W = 4, 256, 64, 64
N = H * W            # 4096 tokens
NCORES = 8
QH = N // 2          # 2048 queries per core
CT = C // 128        # 2 channel tiles
KC = N // 512        # 8 key chunks of 512
NSB = QH // 512      # 4 query superblocks per core
NKT = N // 128       # 32 key tiles
NG = NKT // 2        # 16 key-tile pairs

_CACHE = {}


def _build():
    import concourse.bass as bass
    import concourse.mybir as mybir
    import concourse.tile as tile
    from concourse import bacc

    f32 = mybir.dt.float32
    f16 = mybir.dt.float16
    f8 = mybir.dt.float8e4
    DR = mybir.MatmulPerfMode.DoubleRow
    AF = mybir.ActivationFunctionType

    nc = bacc.Bacc("TRN2", target_bir_lowering=False, debug=True)

    WARM = nc.dram_tensor("warm", [128, 64], f32, kind="ExternalOutput")
    XQ = nc.dram_tensor("xq", [C, QH], f16, kind="ExternalInput")
    XC = nc.dram_tensor("xc", [C, N], f16, kind="ExternalInput")
    WQT = nc.dram_tensor("wqt", [C, C], f16, kind="ExternalInput")
    WKT = nc.dram_tensor("wkt", [C, C], f16, kind="ExternalInput")
    WVT = nc.dram_tensor("wvt", [C, C], f16, kind="ExternalInput")
    WFT = nc.dram_tensor("wft", [2 * C, C], f16, kind="ExternalInput")
    BQ = nc.dram_tensor("bq", [C], f32, kind="ExternalInput")
    BK = nc.dram_tensor("bk", [C], f32, kind="ExternalInput")
    BF = nc.dram_tensor("bf", [C], f32, kind="ExternalInput")
    OUT = nc.dram_tensor("out", [C, QH], f32, kind="ExternalOutput")

    xq_d = XQ.ap().rearrange("(t p) n -> p t n", p=128)
    xc_d = XC.ap().rearrange("(t p) n -> p t n", p=128)
    wq_d = WQT.ap().rearrange("(t p) d -> p t d", p=128)
    wk_d = WKT.ap().rearrange("(t p) d -> p t d", p=128)
    wv_d = WVT.ap().rearrange("(t p) d -> p t d", p=128)
    wf_d = WFT.ap().rearrange("(t p) d -> p t d", p=128)
    out_d = OUT.ap().rearrange("(t p) n -> p t n", p=128)

    with tile.TileContext(nc) as tc:
        with tc.tile_pool(name="persist", bufs=1) as per, \
             tc.tile_pool(name="ptp", bufs=2) as ptp, \
             tc.tile_pool(name="norm", bufs=2) as nrp, \
             tc.tile_pool(name="outp", bufs=2) as outp, \
             tc.tile_pool(name="sc", bufs=3, space="PSUM") as scp, \
             tc.tile_pool(name="rs", bufs=1, space="PSUM") as rsp, \
             tc.tile_pool(name="av", bufs=2, space="PSUM") as avp:

            # ---- persistent tiles ----
            xq_sb = per.tile([128, CT, QH], f16)
            xc_sb = per.tile([128, CT, N], f16)
            wq_sb = per.tile([128, CT, C], f16)
            wk_sb = per.tile([128, CT, C], f16)
            wv_sb = per.tile([128, CT, C], f16)
            wf_sb = per.tile([128, 2 * CT, C], f16)
            bq_sb = per.tile([128, CT], f32)
            bk_sb = per.tile([128, CT], f32)
            bf_sb = per.tile([128, CT], f32)
            q_sb = per.tile([128, CT, QH], f8)
            k_sb = per.tile([128, CT, N], f8)
            vt_sb = per.tile([128, NKT, C], f8)
            ones_sb = per.tile([128, 2, 128], f8)
            onesw_sb = per.tile([128, 128], f16)

            nc.sync.dma_start(wq_sb[:], wq_d)
            nc.sync.dma_start(wk_sb[:], wk_d)
            nc.sync.dma_start(wv_sb[:], wv_d)
            nc.sync.dma_start(wf_sb[:], wf_d)
            nc.sync.dma_start(bq_sb[:], BQ.ap().rearrange("(t p) -> p t", p=128))
            nc.sync.dma_start(bk_sb[:], BK.ap().rearrange("(t p) -> p t", p=128))
            nc.sync.dma_start(bf_sb[:], BF.ap().rearrange("(t p) -> p t", p=128))
            nc.vector.memset(ones_sb[:], 1.0)
            nc.vector.memset(onesw_sb[:], 1.0)

            # ---- PE warm-up: keep the PE busy from t~0 so the HAM clock
            # gate opens (1.2 -> 2.4 GHz) before the real matmuls start.
            wps = scp.tile([128, 512], f32, tag="sc")
            for _ in range(90):
                nc.tensor.matmul(wps[:, :64], onesw_sb[:], onesw_sb[:, :64],
                                 start=True, stop=True)
            w_sb = per.tile([128, 64], f32)
            nc.scalar.activation(w_sb[:], wps[:, :64], AF.Copy)
            nc.sync.dma_start(WARM.ap(), w_sb[:])

            # ---- inputs + projections, interleaved per 512-column chunk ----
            # Q projection: Q[d, n], bias added on DVE
            for qc in range(QH // 512):
                s = slice(qc * 512, (qc + 1) * 512)
                for ct in range(CT):
                    nc.sync.dma_start(xq_sb[:, ct, s], xq_d[:, ct, s])
                for dt in range(CT):
                    ps = scp.tile([128, 512], f32, tag="sc")
                    for ct in range(CT):
                        nc.tensor.matmul(
                            ps[:], wq_sb[:, ct, dt * 128:(dt + 1) * 128],
                            xq_sb[:, ct, s],
                            start=(ct == 0), stop=(ct == CT - 1))
                    nc.scalar.activation(
                        q_sb[:, dt, s], ps[:], AF.Identity,
                        bias=bq_sb[:, dt:dt + 1])

            # K + V^T projections per xc chunk
            for kc in range(KC):
                s = slice(kc * 512, (kc + 1) * 512)
                for ct in range(CT):
                    nc.sync.dma_start(xc_sb[:, ct, s], xc_d[:, ct, s])
                for dt in range(CT):
                    ps = scp.tile([128, 512], f32, tag="sc")
                    for ct in range(CT):
                        nc.tensor.matmul(
                            ps[:], wk_sb[:, ct, dt * 128:(dt + 1) * 128],
                            xc_sb[:, ct, s],
                            start=(ct == 0), stop=(ct == CT - 1))
                    nc.scalar.activation(
                        k_sb[:, dt, s], ps[:], AF.Identity,
                        bias=bk_sb[:, dt:dt + 1])
                # V^T: [keys, d] (no bias; folded into conv bias)
                for j in range(4):
                    kt = 4 * kc + j
                    ps = scp.tile([128, 512], f32, tag="sc")
                    for ct in range(CT):
                        nc.tensor.matmul(
                            ps[:, :C], xc_sb[:, ct, kt * 128:(kt + 1) * 128],
                            wv_sb[:, ct],
                            start=(ct == 0), stop=(ct == CT - 1))
                    nc.vector.tensor_copy(vt_sb[:, kt], ps[:, :C])

            # ---- attention + fused conv, per 512-query superblock ----
            # Per key-tile pair g: two fp8-DR score matmuls + exps feed the
            # rowsum + attended matmuls of pair g-1 (one-pair lag keeps the
            # PE from waiting on ACT). Previous superblock's normalize/conv
            # are emitted after this superblock's first pairs.
            nc._v7_tiles = (outp, scp, wf_sb, xq_sb, bf_sb, out_d)
            prev = None
            for sb in range(NSB):
                qs = slice(sb * 512, (sb + 1) * 512)
                pt = ptp.tile([128, NKT, 512], f8, tag="pt")
                rs = rsp.tile([128, 512], f32, tag="rs")
                aps = avp.tile([128, 2, 512], f32, tag="av")

                def scores_pair(g):
                    for j in range(2):
                        kt = 2 * g + j
                        ps = scp.tile([128, 512], f32, tag="sc")
                        nc.tensor.matmul(
                            ps[:], k_sb[:, :, kt * 128:(kt + 1) * 128],
                            q_sb[:, :, qs],
                            start=True, stop=True, perf_mode=DR)
                        nc.scalar.activation(pt[:, kt], ps[:], AF.Exp)

                def consume_pair(g):
                    for dt in range(CT):
                        nc.tensor.matmul(
                            aps[:, dt],
                            vt_sb[:, 2 * g:2 * (g + 1),
                                  dt * 128:(dt + 1) * 128],
                            pt[:, 2 * g:2 * (g + 1)],
                            start=(g == 0), stop=(g == NG - 1),
                            perf_mode=DR)

                scores_pair(0)
                scores_pair(1)
                if prev is not None:
                    _finish_sb(nc, mybir, prev)
                    prev = None
                for g in range(2, NG):
                    scores_pair(g)
                    consume_pair(g - 2)
                consume_pair(NG - 2)
                consume_pair(NG - 1)

                # rowsum block: constant ones stationary -> single LDW
                for g in range(NG):
                    nc.tensor.matmul(
                        rs[:], ones_sb[:], pt[:, 2 * g:2 * (g + 1)],
                        start=(g == 0), stop=(g == NG - 1), perf_mode=DR)

                rinv = nrp.tile([128, 512], f32, tag="rinv")
                if sb < NSB - 1:
                    nc.vector.reciprocal_approx_fast(rinv[:], rs[:])
                else:
                    # split final reciprocal so the tail chain starts earlier
                    nc.vector.reciprocal_approx_fast(rinv[:, :256],
                                                     rs[:, :256])
                    nc.vector.reciprocal_approx_fast(rinv[:, 256:],
                                                     rs[:, 256:])
                prev = dict(qs=qs, aps=aps, rinv=rinv)

            _finish_sb(nc, mybir, prev, halves=True)
    nc.finalize()
    return nc


def _finish_sb(nc, mybir, env, halves=False):
    """Normalize attended output, fused 1x1 conv, bias, DMA out."""
    f32 = mybir.dt.float32
    f16 = mybir.dt.float16
    tiles = nc._v7_tiles
    qs, aps, rinv = env["qs"], env["aps"], env["rinv"]
    (outp, scp, wf_sb, xq_sb, bf_sb, out_d) = tiles

    a_sb = outp.tile([128, CT, 512], f16, tag="a")
    o_sb = outp.tile([128, CT, 512], f32, tag="o")
    hs = ((slice(0, 256), slice(256, 512)) if halves
          else (slice(0, 512),))
    for h in hs:
        hq = slice(qs.start + h.start, qs.start + h.stop)
        for dt in range(CT):
            nc.vector.tensor_mul(a_sb[:, dt, h], aps[:, dt, h], rinv[:, h])
        for dt in range(CT):
            ops = scp.tile([128, 512], f32, tag="sc")
            for j in range(2 * CT):
                rhs = (xq_sb[:, j, hq] if j < CT
                       else a_sb[:, j - CT, h])
                nc.tensor.matmul(
                    ops[:, h], wf_sb[:, j, dt * 128:(dt + 1) * 128],
                    rhs, start=(j == 0), stop=(j == 2 * CT - 1))
            nc.vector.tensor_scalar_add(
                o_sb[:, dt, h], ops[:, h], bf_sb[:, dt:dt + 1])
            nc.sync.dma_start(out_d[:, dt, hq], o_sb[:, dt, h])


def _get_nc():
    if "nc" not in _CACHE:
        _CACHE["nc"] = _build()
    return _CACHE["nc"]


def _in_maps(transformer_features, cnn_features, Wq, bq, Wk, bk, Wv, bv, Wf, bf):
    xt = np.ascontiguousarray(np.asarray(transformer_features, np.float32)
                              .reshape(B, C, N))
    xc = np.ascontiguousarray(np.asarray(cnn_features, np.float32)
                              .reshape(B, C, N))
    Wq = np.asarray(Wq, np.float32)
    Wk = np.asarray(Wk, np.float32)
    Wv = np.asarray(Wv, np.float32)
    Wf = np.asarray(Wf, np.float32)
    bq = np.asarray(bq, np.float32)
    bk = np.asarray(bk, np.float32)
    bv = np.asarray(bv, np.float32)
    bf = np.asarray(bf, np.float32)

    wqt = np.ascontiguousarray(Wq.T / 16.0).astype(np.float16)
    wkt = np.ascontiguousarray(Wk.T).astype(np.float16)
    wvt = np.ascontiguousarray(Wv.T).astype(np.float16)
    wft = np.ascontiguousarray(Wf.T).astype(np.float16)
    bq_s = bq / 16.0
    bf2 = bf + Wf[:, C:] @ bv
    xt16 = xt.astype(np.float16)
    xc16 = xc.astype(np.float16)

    maps = []
    for c in range(NCORES):
        b, h = divmod(c, 2)
        maps.append(dict(
            xq=np.ascontiguousarray(xt16[b][:, h * QH:(h + 1) * QH]),
            xc=xc16[b],
            wqt=wqt, wkt=wkt, wvt=wvt, wft=wft,
            bq=bq_s, bk=bk, bf=bf2,
        ))
    return maps


def _run(inputs, trace=False):
    from concourse.bass_utils import run_bass_kernel_spmd
    nc = _get_nc()
    maps = _in_maps(**inputs)
    return run_bass_kernel_spmd(nc, maps, list(range(NCORES)), trace=trace)


def kernel(**inputs) -> np.ndarray:
    res = _run(inputs).results
    out = np.empty((B, C, N), np.float32)
    for c in range(NCORES):
        b, h = divmod(c, 2)
        out[b][:, h * QH:(h + 1) * QH] = res[c]["out"]
    return out.reshape(B, C, H, W)
